# revision 40
# baseline (speedup 1.0000x reference)
"""AttentiveDecisionTree Bass kernel for 8 TRN2 NeuronCores.

Sharding: data-parallel over batch (512 rows/core) for everything except the
sparsemax-tau Newton solve, which is sharded over the 1536 (tree,depth) rows
(192/core) and AllGathered (768 B payload).

Algorithm notes (mirrors reference.py):
  - attention: only row s=0 of the MHA output is consumed, so we fold
    mem_ext = [1; memory] into Wk/Wv (Wk2[f,(t,hk)] = mem_ext[t,f]*Wk[f,hk]),
    compute q0/kk/vv as matmuls against x^T, softmax over t=33, project with Wo.
  - sparsemax(z) = relu(z - tau) where tau solves sum(relu(z - tau)) = 1;
    solved with Newton iterations (exact after ~6; we run 8):
        tau <- tau + (sum(relu(z-tau)) - 1) / #{z > tau},  tau0 = (sum(z)-1)/d.
  - odt: bins b = clip(0.5*(feat-thr)*e^{-lt} + 0.5, 0, 1); with a = relu(s*feat+c),
    f0 = relu(1-a), f1 = min(a,1) (f0 = 1-b exactly, f1 = b).  leaf = Kronecker
    product over depth: leaf[l] = hi[l>>3]*lo[l&7] with hi/lo 3-bit products.
    out[b,u] = sum_{n,l} leaf * response  via PE matmuls contracting trees.
All matmuls bf16 with fp32 PSUM accumulation (validated rel_err ~6e-3 < 2e-2).
"""
import os
import sys
from contextlib import ExitStack

import numpy as np

for _p in ("/opt/trn_rl_repo", "/root/.axon_site/_ro/trn_rl_repo"):
    if os.path.isdir(_p) and _p not in sys.path:
        sys.path.append(_p)

import concourse.bass as bass
import concourse.bacc as bacc
import concourse.tile as tile
import concourse.masks as masks
from concourse.tile_rust import add_dep_helper
from concourse import mybir
from concourse.bass_utils import run_bass_kernel_spmd

F32 = mybir.dt.float32
BF16 = mybir.dt.bfloat16
OP = mybir.AluOpType
ACTF = mybir.ActivationFunctionType

NCORES = 8
B, F = 4096, 256
BC = B // NCORES          # 512 batch rows per core
M = 32
S = M + 1                 # 33 attention positions
H, K = 4, 2
HK = H * K                # 8
NT, DEPTH, UNITS = 256, 6, 16
L = 2 ** DEPTH            # 64
ROWS = NT * DEPTH         # 1536 (tree,depth) rows, stored d-major: r = d*256+n
RSH = ROWS // NCORES      # 192 newton rows per core
NJ = ROWS // 128          # 12 row chunks; chunk j = (d=j//2, half=j%2)
NEWTON_ITERS = 8
DEBUG = False
RG = [[list(range(NCORES))]]  # replica groups


def _build_program():
    nc = bacc.Bacc("TRN2", target_bir_lowering=False, debug=False,
                   num_devices=NCORES)

    # ---- DRAM I/O (per-core shapes; host supplies per-core slices) ----
    x_in = nc.dram_tensor("x_in", [BC, F], F32, kind="ExternalInput")
    z_all = nc.dram_tensor("z_all", [ROWS, F], F32, kind="ExternalInput")
    z_sh = nc.dram_tensor("z_sh", [RSH, F], F32, kind="ExternalInput")
    mem_d = nc.dram_tensor("mem", [M, F], F32, kind="ExternalInput")
    wq_d = nc.dram_tensor("wq", [F, HK], F32, kind="ExternalInput")
    wk_d = nc.dram_tensor("wk", [F, HK], F32, kind="ExternalInput")
    wv_d = nc.dram_tensor("wv", [F, HK], F32, kind="ExternalInput")
    bq_d = nc.dram_tensor("bq", [HK], F32, kind="ExternalInput")
    bk_d = nc.dram_tensor("bk", [HK], F32, kind="ExternalInput")
    bv_d = nc.dram_tensor("bv", [HK], F32, kind="ExternalInput")
    wo_d = nc.dram_tensor("wo", [HK, F], F32, kind="ExternalInput")
    bo_d = nc.dram_tensor("bo", [F], F32, kind="ExternalInput")
    th_d = nc.dram_tensor("th", [NT, DEPTH], F32, kind="ExternalInput")
    lt_d = nc.dram_tensor("lt", [NT, DEPTH], F32, kind="ExternalInput")
    resp_d = nc.dram_tensor("resp", [NT, L * UNITS], F32, kind="ExternalInput")
    y_d = nc.dram_tensor("y", [BC, UNITS], F32, kind="ExternalOutput")
    if DEBUG:
        dbg_tau = nc.dram_tensor("dbg_tau", [128, NJ], F32, kind="ExternalOutput")
        dbg_x = nc.dram_tensor("dbg_x", [2, 128, BC], BF16, kind="ExternalOutput")
        dbg_f1 = nc.dram_tensor("dbg_f1", [128, NJ, BC], BF16, kind="ExternalOutput")
        dbg_fs = nc.dram_tensor("dbg_fs", [128, NJ, F], BF16, kind="ExternalOutput")
    tau_in = nc.dram_tensor("tau_in", [RSH], F32)
    tau_out = nc.dram_tensor("tau_out", [ROWS], F32, addr_space="Shared")

    with tile.TileContext(nc) as tc, ExitStack() as ctx:
        cpool = ctx.enter_context(tc.tile_pool(name="const", bufs=1))
        zpool = ctx.enter_context(tc.tile_pool(name="zp", bufs=1))
        fpool = ctx.enter_context(tc.tile_pool(name="fp", bufs=1))
        apool = ctx.enter_context(tc.tile_pool(name="ap", bufs=3))
        hpool = ctx.enter_context(tc.tile_pool(name="hp", bufs=2))
        lpool = ctx.enter_context(tc.tile_pool(name="leaf", bufs=6))
        rpool = ctx.enter_context(tc.tile_pool(name="resp", bufs=2))
        opool = ctx.enter_context(tc.tile_pool(name="outp", bufs=4))
        ps_g = ctx.enter_context(tc.tile_pool(name="ps_g", bufs=1, space="PSUM"))
        ps_x = ctx.enter_context(tc.tile_pool(name="ps_x", bufs=1, space="PSUM"))
        ps_at = ctx.enter_context(tc.tile_pool(name="ps_at", bufs=1, space="PSUM"))
        ps_b = ctx.enter_context(tc.tile_pool(name="ps_b", bufs=1, space="PSUM"))
        ps_a = ctx.enter_context(tc.tile_pool(name="ps_a", bufs=1, space="PSUM"))

        # ================= constant / input loads =================
        # every setup-phase instruction incs setup_sem(+16); the barrier
        # below does a single wait_ge instead of one wait per producer proc
        # (the per-instruction sync-wait encoding limit is ~8).
        _gp_insts, _ve_insts = [], []

        def _setup(inst):
            return inst

        def _gp(inst):
            _gp_insts.append(inst)
            return inst

        def _ve(inst):
            _ve_insts.append(inst)
            return inst

        def make_ident(t):
            _gp(nc.gpsimd.memset(t[:], 0.0))
            _gp(nc.gpsimd.affine_select(
                out=t[:], in_=t[:], compare_op=OP.not_equal, fill=1.0,
                base=0, pattern=[[-1, t.shape[0]]], channel_multiplier=1))

        ident = cpool.tile([128, 128], F32, tag="identf")
        make_ident(ident)
        identb = cpool.tile([128, 128], BF16, tag="identb")
        make_ident(identb)

        ones1 = cpool.tile([1, BC], BF16, tag="ones1")
        _gp(nc.gpsimd.memset(ones1[:], 1.0))

        ix, ix_dmas = [], []
        for i in range(4):
            t = cpool.tile([128, F], F32, tag=f"ix{i}")
            ix_dmas.append(_setup(nc.sync.dma_start(
                t[:], x_in.ap()[i * 128:(i + 1) * 128, :])))
            ix.append(t)

        zs0 = cpool.tile([128, F], F32, tag="zs0")
        zs1 = cpool.tile([64, F], F32, tag="zs1")
        _setup(nc.sync.dma_start(zs0[:], z_sh.ap()[0:128, :]))
        _setup(nc.sync.dma_start(zs1[:], z_sh.ap()[128:RSH, :]))

        mem_sb = cpool.tile([M, F], F32, tag="mem")
        mem_dma = _setup(nc.sync.dma_start(mem_sb[:], mem_d.ap()))

        def load_fw(name, dram):
            # [F, HK] weight -> SBUF [128, 2, HK]; slice [:, h, :] = rows h*128+
            t = cpool.tile([128, 2, HK], F32, tag=name)
            _setup(nc.sync.dma_start(t[:], dram.ap().rearrange("(h p) k -> p h k", p=128)))
            return t

        def load_small(name, dram, shape):
            t = cpool.tile(shape, F32, tag=name)
            ap = dram.ap()
            if len(shape) == 2 and len(ap.shape) == 1:
                ap = ap.rearrange("(a b) -> a b", a=shape[0])
            _setup(nc.sync.dma_start(t[:], ap))
            return t

        wq_f = load_fw("wq", wq_d)
        wk_f = load_fw("wk", wk_d)
        wv_f = load_fw("wv", wv_d)
        wo_f = load_small("wo", wo_d, [HK, F])
        bo_f = load_small("bo", bo_d, [1, F])
        bq_f = load_small("bq", bq_d, [1, HK])
        bk_f = load_small("bk", bk_d, [1, HK])
        bv_f = load_small("bv", bv_d, [1, HK])

        th_t = cpool.tile([128, DEPTH, 2], F32, tag="th")
        _setup(nc.sync.dma_start(th_t[:], th_d.ap().rearrange("(h p) d -> p d h", p=128)))
        lt_t = cpool.tile([128, DEPTH, 2], F32, tag="lt")
        _setup(nc.sync.dma_start(lt_t[:], lt_d.ap().rearrange("(h p) d -> p d h", p=128)))

        # bf16 weight copies
        wq_b = cpool.tile([128, 2, HK], BF16, tag="wqb")
        wo_b = cpool.tile([HK, F], BF16, tag="wob")
        bo_b = cpool.tile([1, F], BF16, tag="bob")
        for dst, src in ((wq_b, wq_f), (wo_b, wo_f), (bo_b, bo_f)):
            _ve(nc.vector.tensor_copy(dst[:], src[:]))
        bq_row = cpool.tile([1, HK], BF16, tag="bqrow")
        _ve(nc.vector.tensor_copy(bq_row[:], bq_f[:]))
        bk_row = cpool.tile([1, S * HK], BF16, tag="bkrow")
        bv_row = cpool.tile([1, S * HK], BF16, tag="bvrow")
        _ve(nc.vector.tensor_copy(
            bk_row[:].rearrange("a (t k) -> a t k", t=S),
            bk_f[:].unsqueeze(1).broadcast_to((1, S, HK))))
        _ve(nc.vector.tensor_copy(
            bv_row[:].rearrange("a (t k) -> a t k", t=S),
            bv_f[:].unsqueeze(1).broadcast_to((1, S, HK))))

        # pre-barrier PE warm-up: observe the Pool proc (identities) before
        # the barrier exists, so the first post-barrier PE instruction only
        # needs the SP wait (matmul encoding allows a single sem wait).
        pwarm0 = ps_b.tile([1, 64], F32, tag="pwarm")
        _pe_warm = nc.tensor.transpose(pwarm0[0:1, 0:1], ident[0:1, 0:1],
                                       ident[0:1, 0:1])
        for si in _gp_insts:
            add_dep_helper(_pe_warm.ins, si.ins, reason="pe pre-warm")


        # (no global forward-edge barrier: per-phase PE absorbers handle the
        # matmul 1-wait encoding limit; other engines tolerate direct waits)

        # PE wait absorbers: the matmul/transpose (S3_LW) encoding carries at
        # most ONE semaphore wait, and only real PE instructions advance PE's
        # observed vector clock.  Before each PE phase we emit tiny 1x1
        # transposes, each depending on producers from a single engine proc,
        # so the phase's matmuls are left with <=1 un-observed producer.
        pwarm = pwarm0
        pw_n = [1]

        def pe_absorb(producers):
            c = pw_n[0] % 64
            pw_n[0] += 1
            t = nc.tensor.transpose(pwarm[0:1, c:c + 1], ident[0:1, 0:1],
                                    ident[0:1, 0:1])
            for p in producers:
                add_dep_helper(t.ins, p.ins, reason="pe wait absorb")
            return t

        pe_absorb([])              # absorbs the barrier's SP tick
        pe_absorb(_gp_insts)       # Pool: identities + ones
        for d in ix_dmas:
            pe_absorb([d])         # each input DMA queue
        pe_absorb([mem_dma])

        # big loads (post-barrier so the barrier doesn't wait on them)
        zt = cpool.tile([128, NJ, F], F32, tag="zt")
        z_view = z_all.ap().rearrange("(j p) f -> j p f", p=128)
        for j in range(NJ):
            nc.sync.dma_start(zt[:, j, :], z_view[j])
        resps, respb_insts = [], []
        for h in range(2):
            respf = rpool.tile([128, L * UNITS], F32, tag="respf",
                               name=f"respf{h}")
            nc.sync.dma_start(respf[:], resp_d.ap()[h * 128:(h + 1) * 128, :])
            respb = rpool.tile([128, L * UNITS], BF16, tag="respb",
                               name=f"respb{h}")
            respb_insts.append(nc.scalar.copy(respb[:], respf[:]))
            resps.append(respb)

        # ================= Newton sparsemax on 192-row shard =================
        nst = cpool.tile([128, 8], F32, tag="nst")  # s0,k0,rk0,d0 | s1,k1,rk1,d1
        tau0 = cpool.tile([128, 1], F32, tag="tau0")
        tau1 = cpool.tile([64, 1], F32, tag="tau1")
        ntau0 = cpool.tile([128, 1], F32, tag="ntau0")
        ntau1 = cpool.tile([64, 1], F32, tag="ntau1")
        scr = cpool.tile([128, F], BF16, tag="nscr")
        scra = cpool.tile([128, F], BF16, tag="nscra")

        s0, k0 = nst[:, 0:1], nst[:, 1:2]
        rk0, d0 = nst[:, 2:3], nst[:, 3:4]
        s1, k1 = nst[0:64, 4:5], nst[0:64, 5:6]
        rk1, d1 = nst[0:64, 6:7], nst[0:64, 7:8]

        nc.vector.tensor_reduce(s0, zs0[:], mybir.AxisListType.X, OP.add)
        nc.vector.tensor_scalar(tau0[:], s0, -1.0, 1.0 / F, OP.add, OP.mult)
        nc.vector.tensor_scalar(ntau0[:], tau0[:], -1.0, None, OP.mult)
        nc.vector.tensor_reduce(s1, zs1[:], mybir.AxisListType.X, OP.add)
        nc.vector.tensor_scalar(tau1[:], s1, -1.0, 1.0 / F, OP.add, OP.mult)
        nc.vector.tensor_scalar(ntau1[:], tau1[:], -1.0, None, OP.mult)

        # NOTE tensor_scalar's accum_out reduces with op1 (not sum), so the
        # relu+sum uses the ACT engine: accum_out there is a plain sum.
        for _ in range(NEWTON_ITERS):
            nc.scalar.activation(scra[:], zs0[:], ACTF.Relu,
                                 bias=ntau0[:], accum_out=s0)
            nc.vector.tensor_scalar(scr[:], zs0[:], tau0[:], 0.0,
                                    OP.is_gt, OP.add, accum_out=k0)
            nc.vector.reciprocal(rk0, k0)
            nc.vector.tensor_scalar(d0, s0, -1.0, rk0, OP.add, OP.mult)
            nc.vector.scalar_tensor_tensor(tau0[:], d0, 1.0, tau0[:],
                                           OP.mult, OP.add)
            nc.vector.tensor_scalar(ntau0[:], tau0[:], -1.0, None, OP.mult)
            nc.scalar.activation(scra[0:64, :], zs1[:], ACTF.Relu,
                                 bias=ntau1[:], accum_out=s1)
            nc.vector.tensor_scalar(scr[0:64, :], zs1[:], tau1[:], 0.0,
                                    OP.is_gt, OP.add, accum_out=k1)
            nc.vector.reciprocal(rk1, k1)
            nc.vector.tensor_scalar(d1, s1, -1.0, rk1, OP.add, OP.mult)
            nc.vector.scalar_tensor_tensor(tau1[:], d1, 1.0, tau1[:],
                                           OP.mult, OP.add)
            nc.vector.tensor_scalar(ntau1[:], tau1[:], -1.0, None, OP.mult)

        # allgather tau
        nc.sync.dma_start(tau_in.ap()[0:128], tau0[:])
        nc.sync.dma_start(tau_in.ap()[128:RSH], tau1[:])
        nc.gpsimd.collective_compute(
            "AllGather", OP.bypass, replica_groups=RG[0],
            ins=[tau_in.ap()], outs=[tau_out.ap()])
        tauall = cpool.tile([128, NJ], F32, tag="tauall")
        nc.sync.dma_start(tauall[:], tau_out.ap().rearrange("(j p) -> p j", p=128))

        # ================= input transpose x^T (PE) =================
        inT = []
        inTb = [[None] * 4, [None] * 4]
        inTb_copies = []
        act_evicts = []
        for h in range(2):
            tT = cpool.tile([128, BC], F32, tag=f"inT{h}", name=f"inT{h}")
            inT.append(tT)
        for i in range(4):
            for h in range(2):
                pt = ps_g.tile([128, 512], F32, tag="psg", name="pt")
                nc.tensor.transpose(pt[:, 0:128], ix[i][:, h * 128:(h + 1) * 128], ident[:])
                act_evicts.append(nc.scalar.copy(
                    inT[h][:, i * 128:(i + 1) * 128], pt[:, 0:128]))
                tb = cpool.tile([128, 128], BF16, tag=f"inTb{h}_{i}", name=f"inTb{h}_{i}")
                inTb_copies.append(nc.vector.tensor_copy(
                    tb[:], inT[h][:, i * 128:(i + 1) * 128]))
                inTb[h][i] = tb

        # ================= attention =================
        # mem_ext^T [128f, 33] per f-half; col 0 = ones
        memT = []
        for h in range(2):
            t = cpool.tile([128, S], F32, tag=f"memT{h}")
            pt = ps_g.tile([128, 512], F32, tag="psg", name="pt")
            nc.tensor.transpose(pt[0:128, 0:M], mem_sb[:, h * 128:(h + 1) * 128],
                                ident[0:M, 0:M])
            nc.gpsimd.memset(t[:, 0:1], 1.0)
            act_evicts.append(nc.scalar.copy(t[:, 1:S], pt[0:128, 0:M]))
            memT.append(t)

        # folded K/V weights: wk2[f, t, hk] = memExtT[f, t] * wk[f, hk]
        # DVE absorber: observe the ACT (memT eviction) tick first so the
        # tensor_tensor ops stay within the DVE wait-encoding limit.
        dve_scr = cpool.tile([1, 4], F32, tag="dvescr")
        ab_dve = nc.vector.tensor_copy(dve_scr[0:1, 0:1], bq_row[0:1, 0:1])
        for _ae in act_evicts:
            add_dep_helper(ab_dve.ins, _ae.ins, reason="dve absorb act")
        wk2, wv2 = [], []
        attn_dve = list(_ve_insts)
        first_w2 = [None]
        for h in range(2):
            for name, wsrc, dstl in (("k", wk_f, wk2), ("v", wv_f, wv2)):
                t = cpool.tile([128, S, HK], BF16, tag=f"w2{name}{h}")
                _w2i = nc.vector.tensor_tensor(
                    t[:],
                    memT[h][:].unsqueeze(2).broadcast_to((128, S, HK)),
                    wsrc[:, h, :].unsqueeze(1).broadcast_to((128, S, HK)),
                    OP.mult)
                add_dep_helper(_w2i.ins, ab_dve.ins, sync=False,
                               reason="order after dve absorber")
                attn_dve.append(_w2i)
                dstl.append(t)

        pe_absorb(attn_dve + inTb_copies)

        # per 128-batch chunk: q0, kk, vv, softmax, o
        obb = []  # normalized attention outputs [128, HK] bf16, per chunk
        obb_insts = []
        prev_act = list(act_evicts)
        prev_dve = []
        for i in range(4):
            ab1 = pe_absorb(prev_act)
            ab2 = pe_absorb(prev_dve)
            prev_act, prev_dve = [], []
            xTc = [inTb[h][i][:] for h in range(2)]
            kvA = ps_at.tile([128, 512], F32, tag="kvA", name="kvA")
            for h in range(2):
                mm = nc.tensor.matmul(kvA[:, 432:432 + HK], xTc[h],
                                      wq_b[:, h, :],
                                      start=(h == 0), stop=False)
                if h == 0:
                    add_dep_helper(mm.ins, ab1.ins, sync=False,
                                   reason="order after absorber")
                    add_dep_helper(mm.ins, ab2.ins, sync=False,
                                   reason="order after absorber")
            nc.tensor.matmul(kvA[:, 432:432 + HK],
                             ones1[:, i * 128:(i + 1) * 128],
                             bq_row[:], start=False, stop=True)
            kkp = kvA
            vvp = ps_at.tile([128, 512], F32, tag="kvB", name="kvB")
            for dst, w2, brow in ((kkp, wk2, bk_row), (vvp, wv2, bv_row)):
                for h in range(2):
                    nc.tensor.matmul(dst[:, 0:S * HK], xTc[h],
                                     w2[h][:].rearrange("p t k -> p (t k)"),
                                     start=(h == 0), stop=False)
                nc.tensor.matmul(dst[:, 0:S * HK], ones1[:, i * 128:(i + 1) * 128],
                                 brow[:], start=False, stop=True)

            q0s = apool.tile([128, HK], F32, tag="q0s")
            prev_act.append(nc.scalar.copy(q0s[:], kvA[:, 432:432 + HK]))
            # scores: sum_k q0*kk -> sc_ht [128, h, t]
            prod = apool.tile([128, S, H, K], F32, tag="prod")
            prev_dve.append(nc.vector.tensor_tensor(
                prod[:], kkp[:, 0:S * HK].rearrange("p (t h k) -> p t h k", t=S, h=H),
                q0s[:].rearrange("p (h k) -> p h k", h=H).unsqueeze(1)
                    .broadcast_to((128, S, H, K)),
                OP.mult))
            sc_ht = apool.tile([128, H, S], F32, tag="scht")
            prev_dve.append(nc.vector.tensor_reduce(
                sc_ht[:].transpose([0, 2, 1]), prod[:],
                mybir.AxisListType.X, OP.add))
            mx = apool.tile([128, H, 2], F32, tag="mx")
            prev_dve.append(nc.vector.tensor_reduce(
                mx[:, :, 0:1].squeeze(2), sc_ht[:],
                mybir.AxisListType.X, OP.max))
            prev_dve.append(nc.vector.tensor_scalar(
                mx[:, :, 1:2], mx[:, :, 0:1], -(2.0 ** -0.5), None, OP.mult))
            ex = apool.tile([128, H, S], BF16, tag="ex")
            den = apool.tile([128, H, 2], F32, tag="den")
            for hh in range(H):
                prev_act.append(nc.scalar.activation(
                    ex[:, hh, :], sc_ht[:, hh, :], ACTF.Exp,
                    bias=mx[:, hh, 1:2], scale=2.0 ** -0.5,
                    accum_out=den[:, hh, 0:1]))
            po = apool.tile([128, H, K, S], F32, tag="po")
            prev_dve.append(nc.vector.tensor_tensor(
                po[:].transpose([0, 3, 1, 2]),
                vvp[:, 0:S * HK].rearrange("p (t h k) -> p t h k", t=S, h=H),
                ex[:].transpose([0, 2, 1]).unsqueeze(3)
                    .broadcast_to((128, S, H, K)),
                OP.mult))
            ov = apool.tile([128, H, K], F32, tag="ov")
            prev_dve.append(nc.vector.tensor_reduce(
                ov[:], po[:], mybir.AxisListType.X, OP.add))
            prev_dve.append(nc.vector.reciprocal(den[:, :, 1:2],
                                                 den[:, :, 0:1]))
            ob = apool.tile([128, HK], BF16, tag=f"ob{i}")
            _obm = nc.vector.tensor_tensor(
                ob[:].rearrange("p (h k) -> p h k", h=H), ov[:],
                den[:, :, 1:2].broadcast_to((128, H, K)), OP.mult)
            obb_insts.append(_obm)
            prev_dve.append(_obm)
            obb.append(ob)

        # o^T [8, 512] bf16, then xhat^T and x^T bf16
        pe_absorb(prev_act)
        pe_absorb(prev_dve)
        oTb, oTb_cps = [], []
        for i in range(4):
            abs_i = [pe_absorb([obb_insts[i]])]
            if i:
                abs_i.append(pe_absorb([oTb_cps[i - 1]]))
            pt = ps_b.tile([128, 128], BF16, tag="ptrb", name="pt2")
            tri = nc.tensor.transpose(pt[0:HK, 0:128], obb[i][:], identb[:])
            for ab in abs_i:
                add_dep_helper(tri.ins, ab.ins, sync=False,
                               reason="order after absorber")
            t = cpool.tile([HK, 128], BF16, tag=f"oTb{i}", name=f"oTb{i}")
            oTb_cps.append(nc.scalar.copy(t[:], pt[0:HK, 0:128]))
            oTb.append(t)
        ab_oTb = pe_absorb(oTb_cps)

        xTb, xTb_insts = [], []
        for h in range(2):
            xh = ps_x.tile([128, BC], F32, tag="xh", name="xh")
            for i in range(4):
                sl = slice(i * 128, (i + 1) * 128)
                mm = nc.tensor.matmul(xh[:, sl],
                                      wo_b[:, h * 128:(h + 1) * 128],
                                      oTb[i][:], start=True, stop=False)
                if i == 0:
                    add_dep_helper(mm.ins, ab_oTb.ins, sync=False,
                                   reason="order after absorber")
                nc.tensor.matmul(xh[:, sl], bo_b[:, h * 128:(h + 1) * 128],
                                 ones1[:, sl], start=False, stop=True)
            xb = cpool.tile([128, BC], BF16, tag=f"xTb{h}")
            xTb_insts.append(
                nc.vector.tensor_tensor(xb[:], inT[h][:], xh[:], OP.add))
            xTb.append(xb)

        # ================= sparsemax output + transpose =================
        fs_t = cpool.tile([128, NJ, F], BF16, tag="fs")
        for j in range(NJ):
            nc.gpsimd.tensor_scalar(fs_t[:, j, :], zt[:, j, :],
                                    tauall[:, j:j + 1], 0.0,
                                    OP.subtract, OP.max)
        fsT = [[None] * NJ, [None] * NJ]
        fsT_dmas = []
        for j in range(NJ):
            for h in range(2):
                t = cpool.tile([128, 128], BF16, tag=f"fsT{h}_{j}",
                               name=f"fsT{h}_{j}")
                fsT_dmas.append(nc.sync.dma_start_transpose(
                    t[:], fs_t[:, j, h * 128:(h + 1) * 128]))
                fsT[h][j] = t

        if DEBUG:
            nc.sync.dma_start(dbg_tau.ap(), tauall[:])
            nc.sync.dma_start(dbg_fs.ap(), fs_t[:])

        # scale s = 0.5*exp(-lt), offset c = 0.5 - th*s   (both [128, 12])
        sv = cpool.tile([128, NJ], F32, tag="sv")
        cv = cpool.tile([128, NJ], F32, tag="cv")
        lt_flat = lt_t[:].rearrange("p d h -> p (d h)")
        th_flat = th_t[:].rearrange("p d h -> p (d h)")
        nc.scalar.activation(sv[:], lt_flat, ACTF.Exp, scale=-1.0)
        nc.vector.tensor_scalar(sv[:], sv[:], 0.5, None, OP.mult)
        nc.vector.tensor_tensor(cv[:], th_flat, sv[:], OP.mult)
        nc.vector.tensor_scalar(cv[:], cv[:], -1.0, 0.5, OP.mult, OP.add)

        # ================= feat matmul + bins =================
        if DEBUG:
            for h in range(2):
                nc.sync.dma_start(dbg_x.ap()[h], xTb[h][:])
        pe_absorb(xTb_insts)
        for d in fsT_dmas:
            pe_absorb([d])

        f0_t = fpool.tile([128, NJ, BC], BF16, tag="f0")
        f1_t = fpool.tile([128, NJ, BC], BF16, tag="f1")
        aj_hist = []
        for j in range(NJ):
            ab_j = pe_absorb([aj_hist.pop(0)]) if aj_hist else None
            ft = ps_g.tile([128, BC], F32, tag="psg", name="ft")
            for h in range(2):
                mm = nc.tensor.matmul(ft[:], fsT[h][j][:],
                                      xTb[h][:], start=(h == 0),
                                      stop=(h == 1))
                if h == 0 and ab_j is not None:
                    add_dep_helper(mm.ins, ab_j.ins, sync=False,
                                   reason="order after absorber")
            aj = apool.tile([128, BC], BF16, tag="aj")
            aj_hist.append(nc.scalar.activation(
                aj[:], ft[:], ACTF.Relu,
                bias=cv[:, j:j + 1], scale=sv[:, j:j + 1]))
            nc.scalar.activation(f0_t[:, j, :], aj[:], ACTF.Relu,
                                 bias=1.0, scale=-1.0)
            nc.gpsimd.tensor_scalar(f1_t[:, j, :], aj[:], 1.0, None, OP.min)

        if DEBUG:
            nc.sync.dma_start(dbg_f1.ap(), f1_t[:])

        # ================= kronecker + tree contraction =================
        pe_absorb(respb_insts)
        accs = [ps_a.tile([UNITS, BC], F32, tag=f"acc{a}", name=f"acc{a}")
                for a in range(2)]
        for h in range(2):
            respb = resps[h]

            def fbit(d, b):
                t = f1_t if b else f0_t
                return t[:, d * 2 + h, :]

            hi2 = hpool.tile([128, 4, BC], BF16, tag="hi2")
            lo2 = hpool.tile([128, 4, BC], BF16, tag="lo2")
            hi = hpool.tile([128, 8, BC], BF16, tag="hi")
            lo = hpool.tile([128, 8, BC], BF16, tag="lo")
            for i in range(4):
                nc.vector.tensor_tensor(hi2[:, i, :], fbit(5, i >> 1),
                                        fbit(4, i & 1), OP.mult)
                nc.vector.tensor_tensor(lo2[:, i, :], fbit(2, i >> 1),
                                        fbit(1, i & 1), OP.mult)
            for i in range(8):
                nc.vector.tensor_tensor(hi[:, i, :], hi2[:, i >> 1, :],
                                        fbit(3, i & 1), OP.mult)
                nc.vector.tensor_tensor(lo[:, i, :], lo2[:, i >> 1, :],
                                        fbit(0, i & 1), OP.mult)
            for l in range(L):
                leaf = lpool.tile([128, BC], BF16, tag="leaf")
                nc.vector.tensor_tensor(leaf[:], hi[:, l >> 3, :],
                                        lo[:, l & 7, :], OP.mult)
                nc.tensor.matmul(accs[l % 2][:],
                                 respb[:, l * UNITS:(l + 1) * UNITS],
                                 leaf[:],
                                 start=(h == 0 and l < 2),
                                 stop=(h == 1 and l >= L - 2))

        # sum the 4 accumulators, transpose, store
        outT = opool.tile([UNITS, BC], F32, tag="outT")
        nc.vector.tensor_copy(outT[:], accs[0][:])
        last_add = nc.vector.tensor_tensor(outT[:], outT[:], accs[1][:],
                                           OP.add)
        ab_o1 = pe_absorb(aj_hist)
        ab_o2 = pe_absorb([last_add])
        ysb_prev = None
        for i in range(4):
            abs_i = [ab_o1, ab_o2]
            if ysb_prev is not None:
                abs_i.append(pe_absorb([ysb_prev]))
            pt = ps_g.tile([128, 512], F32, tag="psg", name="pt")
            tri = nc.tensor.transpose(pt[:, 0:UNITS],
                                      outT[:, i * 128:(i + 1) * 128],
                                      ident[0:UNITS, 0:UNITS])
            for ab in abs_i:
                add_dep_helper(tri.ins, ab.ins, sync=False,
                               reason="order after absorber")
            ysb = opool.tile([128, UNITS], F32, tag="ysb")
            ysb_prev = nc.scalar.copy(ysb[:], pt[:, 0:UNITS])
            nc.sync.dma_start(y_d.ap()[i * 128:(i + 1) * 128, :], ysb[:])

    nc.compile()
    return nc


_CACHED = None


def _get_program():
    global _CACHED
    if _CACHED is None:
        _CACHED = _build_program()
    return _CACHED


def _make_in_maps(inputs, memory, Wq, bq, Wk, bk, Wv, bv, Wo, bo,
                  fs_logits, thresholds, log_temp, response):
    f32 = np.float32
    z_dmaj = np.ascontiguousarray(
        np.asarray(fs_logits, f32).transpose(1, 0, 2).reshape(ROWS, F))
    common = {
        "z_all": z_dmaj,
        "mem": np.ascontiguousarray(np.asarray(memory, f32)),
        "wq": np.ascontiguousarray(np.asarray(Wq, f32).reshape(F, HK)),
        "wk": np.ascontiguousarray(np.asarray(Wk, f32).reshape(F, HK)),
        "wv": np.ascontiguousarray(np.asarray(Wv, f32).reshape(F, HK)),
        "bq": np.ascontiguousarray(np.asarray(bq, f32).reshape(HK)),
        "bk": np.ascontiguousarray(np.asarray(bk, f32).reshape(HK)),
        "bv": np.ascontiguousarray(np.asarray(bv, f32).reshape(HK)),
        "wo": np.ascontiguousarray(np.asarray(Wo, f32).reshape(HK, F)),
        "bo": np.ascontiguousarray(np.asarray(bo, f32).reshape(F)),
        "th": np.ascontiguousarray(np.asarray(thresholds, f32)),
        "lt": np.ascontiguousarray(np.asarray(log_temp, f32)),
        "resp": np.ascontiguousarray(
            np.asarray(response, f32).reshape(NT, L * UNITS)),
    }
    xs = np.ascontiguousarray(np.asarray(inputs, f32))
    in_maps = []
    for c in range(NCORES):
        m = dict(common)
        m["x_in"] = np.ascontiguousarray(xs[c * BC:(c + 1) * BC])
        m["z_sh"] = np.ascontiguousarray(z_dmaj[c * RSH:(c + 1) * RSH])
        in_maps.append(m)
    return in_maps


def run(inputs_dict, trace=False):
    nc = _get_program()
    in_maps = _make_in_maps(**inputs_dict)
    res = run_bass_kernel_spmd(nc, in_maps, list(range(NCORES)), trace=trace)
    out = np.concatenate([res.results[c]["y"] for c in range(NCORES)], axis=0)
    return out.astype(np.float32), res


def kernel(inputs, memory, Wq, bq, Wk, bk, Wv, bv, Wo, bo,
           fs_logits, thresholds, log_temp, response):
    out, _ = run(dict(
        inputs=inputs, memory=memory, Wq=Wq, bq=bq, Wk=Wk, bk=bk,
        Wv=Wv, bv=bv, Wo=Wo, bo=bo, fs_logits=fs_logits,
        thresholds=thresholds, log_temp=log_temp, response=response))
    return out


# revision 41
# speedup vs baseline: 1.5402x; 1.5402x over previous
"""AttentiveDecisionTree Bass kernel for 8 TRN2 NeuronCores.

Sharding: data-parallel over batch (512 rows/core); the sparsemax-tau Newton
solve is sharded over the 1536 (tree,depth) rows (192/core) and AllGathered
(768 B payload).

Algorithm (mirrors reference.py):
  - attention: only row s=0 of the MHA output is consumed, so mem_ext =
    [1; memory] folds into Wk/Wv (Wk2[f,(t,hk)] = mem_ext[t,f]*Wk[f,hk]);
    q0/kk/vv are matmuls against x^T, softmax over t=33, project with Wo.
  - sparsemax(z) = relu(z - tau), tau solving sum(relu(z - tau)) = 1 via
    Newton (exact after ~6 iters; we run 8):
        tau += (sum(relu(z-tau)) - 1) / #{z > tau},  tau0 = (sum(z)-1)/d.
    The relu+sum rides the ACT engine (activation accum_out sums; DVE
    tensor_scalar's accum_out reduces with op1 instead).
  - odt: with a = relu(s*feat + c), s = 0.5 e^{-lt}, c = 0.5 - thr*s:
    f1 = min(a,1) = bins, f0 = 1 - f1 (exactly).  leaf = Kronecker product:
    leaf[l] = hi[l>>3]*lo[l&7]; out = sum_{n,l} leaf*response via PE matmuls
    contracting the tree dim on partitions.
All matmuls bf16 with fp32 PSUM accumulation (rel_err ~7e-3 < 2e-2).
Rows are stored d-major (r = d*256 + n) so row-chunk j = (d=j//2, half=j%2).
"""
import os
import sys
from contextlib import ExitStack

import numpy as np

for _p in ("/opt/trn_rl_repo", "/root/.axon_site/_ro/trn_rl_repo"):
    if os.path.isdir(_p) and _p not in sys.path:
        sys.path.append(_p)

import concourse.bass as bass
import concourse.bacc as bacc
import concourse.tile as tile
from concourse import mybir
from concourse.bass_utils import run_bass_kernel_spmd

F32 = mybir.dt.float32
BF16 = mybir.dt.bfloat16
OP = mybir.AluOpType
ACTF = mybir.ActivationFunctionType
AX = mybir.AxisListType

NCORES = 8
B, F = 4096, 256
BC = B // NCORES
M = 32
S = M + 1
H, K = 4, 2
HK = H * K
NT, DEPTH, UNITS = 256, 6, 16
L = 2 ** DEPTH
ROWS = NT * DEPTH
RSH = ROWS // NCORES
NJ = ROWS // 128
NEWTON_ITERS = 8
DEBUG = False
RG = [list(range(NCORES))]


def _make_ident(nc, t):
    nc.gpsimd.memset(t[:], 0.0)
    nc.gpsimd.affine_select(
        out=t[:], in_=t[:], compare_op=OP.not_equal, fill=1.0,
        base=0, pattern=[[-1, t.shape[0]]], channel_multiplier=1)


def _build_program():
    nc = bacc.Bacc("TRN2", target_bir_lowering=False, debug=False,
                   num_devices=NCORES)

    x_in = nc.dram_tensor("x_in", [BC, F], F32, kind="ExternalInput")
    z_all = nc.dram_tensor("z_all", [ROWS, F], F32, kind="ExternalInput")
    z_sh = nc.dram_tensor("z_sh", [RSH, F], F32, kind="ExternalInput")
    mem_d = nc.dram_tensor("mem", [M, F], F32, kind="ExternalInput")
    wq_d = nc.dram_tensor("wq", [F, HK], F32, kind="ExternalInput")
    wk_d = nc.dram_tensor("wk", [F, HK], F32, kind="ExternalInput")
    wv_d = nc.dram_tensor("wv", [F, HK], F32, kind="ExternalInput")
    bq_d = nc.dram_tensor("bq", [HK], F32, kind="ExternalInput")
    bk_d = nc.dram_tensor("bk", [HK], F32, kind="ExternalInput")
    bv_d = nc.dram_tensor("bv", [HK], F32, kind="ExternalInput")
    wo_d = nc.dram_tensor("wo", [HK, F], F32, kind="ExternalInput")
    bo_d = nc.dram_tensor("bo", [F], F32, kind="ExternalInput")
    th_d = nc.dram_tensor("th", [NT, DEPTH], F32, kind="ExternalInput")
    lt_d = nc.dram_tensor("lt", [NT, DEPTH], F32, kind="ExternalInput")
    resp_d = nc.dram_tensor("resp", [NT, L * UNITS], F32, kind="ExternalInput")
    y_d = nc.dram_tensor("y", [BC, UNITS], F32, kind="ExternalOutput")
    if DEBUG:
        dbg_tau = nc.dram_tensor("dbg_tau", [128, NJ], F32,
                                 kind="ExternalOutput")
        dbg_x = nc.dram_tensor("dbg_x", [2, 128, BC], BF16,
                               kind="ExternalOutput")
        dbg_f1 = nc.dram_tensor("dbg_f1", [128, NJ, BC], BF16,
                                kind="ExternalOutput")
    tau_in = nc.dram_tensor("tau_in", [RSH], F32)
    tau_out = nc.dram_tensor("tau_out", [ROWS], F32, addr_space="Shared")

    with tile.TileContext(nc) as tc, ExitStack() as ctx:
        cpool = ctx.enter_context(tc.tile_pool(name="const", bufs=1))
        fpool = ctx.enter_context(tc.tile_pool(name="fp", bufs=1))
        apool = ctx.enter_context(tc.tile_pool(name="ap", bufs=3))
        hpool = ctx.enter_context(tc.tile_pool(name="hp", bufs=2))
        lpool = ctx.enter_context(tc.tile_pool(name="leaf", bufs=2))
        rpool = ctx.enter_context(tc.tile_pool(name="resp", bufs=1))
        opool = ctx.enter_context(tc.tile_pool(name="outp", bufs=4))
        ps_g = ctx.enter_context(tc.tile_pool(name="ps_g", bufs=2, space="PSUM"))
        ps_at = ctx.enter_context(tc.tile_pool(name="ps_at", bufs=1, space="PSUM"))
        ps_b = ctx.enter_context(tc.tile_pool(name="ps_b", bufs=2, space="PSUM"))
        ps_a = ctx.enter_context(tc.tile_pool(name="ps_a", bufs=1, space="PSUM"))

        # ---------------- constants & loads ----------------
        ident = cpool.tile([128, 128], F32, tag="identf")
        _make_ident(nc, ident)
        identb = cpool.tile([128, 128], BF16, tag="identb")
        _make_ident(nc, identb)
        ones1 = cpool.tile([1, BC], BF16, tag="ones1")
        nc.gpsimd.memset(ones1[:], 1.0)

        ix = []
        for i in range(4):
            t = cpool.tile([128, F], F32, tag=f"ix{i}", name=f"ix{i}")
            nc.sync.dma_start(t[:], x_in.ap()[i * 128:(i + 1) * 128, :])
            ix.append(t)

        zs0 = cpool.tile([128, F], F32, tag="zs0")
        zs1 = cpool.tile([64, F], F32, tag="zs1")
        nc.sync.dma_start(zs0[:], z_sh.ap()[0:128, :])
        nc.sync.dma_start(zs1[:], z_sh.ap()[128:RSH, :])

        mem_sb = cpool.tile([M, F], F32, tag="mem")
        nc.sync.dma_start(mem_sb[:], mem_d.ap())

        def load_fw(name, dram):
            t = cpool.tile([128, 2, HK], F32, tag=name, name=name)
            nc.sync.dma_start(t[:], dram.ap().rearrange("(h p) k -> p h k",
                                                        p=128))
            return t

        def load_small(name, dram, shape):
            t = cpool.tile(shape, F32, tag=name, name=name)
            ap = dram.ap()
            if len(shape) == 2 and len(ap.shape) == 1:
                ap = ap.rearrange("(a b) -> a b", a=shape[0])
            nc.sync.dma_start(t[:], ap)
            return t

        wq_f = load_fw("wq", wq_d)
        wk_f = load_fw("wk", wk_d)
        wv_f = load_fw("wv", wv_d)
        wo_f = load_small("wo", wo_d, [HK, F])
        bo_f = load_small("bo", bo_d, [1, F])
        bq_f = load_small("bq", bq_d, [1, HK])
        bk_f = load_small("bk", bk_d, [1, HK])
        bv_f = load_small("bv", bv_d, [1, HK])

        # th/lt: natural [128, h, d] load (24 B lines), permuted on-chip to
        # [128, d, h] so that column j = d*2+h matches the row chunks.
        th_n = cpool.tile([128, 2, DEPTH], F32, tag="thn")
        lt_n = cpool.tile([128, 2, DEPTH], F32, tag="ltn")
        nc.sync.dma_start(th_n[:], th_d.ap().rearrange("(h p) d -> p h d",
                                                       p=128))
        nc.sync.dma_start(lt_n[:], lt_d.ap().rearrange("(h p) d -> p h d",
                                                       p=128))
        th_t = cpool.tile([128, DEPTH, 2], F32, tag="th")
        lt_t = cpool.tile([128, DEPTH, 2], F32, tag="lt")
        nc.vector.tensor_copy(th_t[:], th_n[:].transpose([0, 2, 1]))
        nc.vector.tensor_copy(lt_t[:], lt_n[:].transpose([0, 2, 1]))

        wq_b = cpool.tile([128, 2, HK], BF16, tag="wqb")
        wo_b = cpool.tile([HK, F], BF16, tag="wob")
        bo_b = cpool.tile([1, F], BF16, tag="bob")
        bq_row = cpool.tile([1, HK], BF16, tag="bqrow")
        for dst, src in ((wq_b, wq_f), (wo_b, wo_f), (bo_b, bo_f),
                         (bq_row, bq_f)):
            nc.vector.tensor_copy(dst[:], src[:])
        bk_row = cpool.tile([1, S * HK], BF16, tag="bkrow")
        bv_row = cpool.tile([1, S * HK], BF16, tag="bvrow")
        nc.vector.tensor_copy(
            bk_row[:].rearrange("a (t k) -> a t k", t=S),
            bk_f[:].unsqueeze(1).broadcast_to((1, S, HK)))
        nc.vector.tensor_copy(
            bv_row[:].rearrange("a (t k) -> a t k", t=S),
            bv_f[:].unsqueeze(1).broadcast_to((1, S, HK)))

        # big loads
        zt = cpool.tile([128, NJ, F], F32, tag="zt")
        z_view = z_all.ap().rearrange("(j p) f -> j p f", p=128)
        for j in range(NJ):
            nc.sync.dma_start(zt[:, j, :], z_view[j])
        resps = []
        for h in range(2):
            respf = rpool.tile([128, L * UNITS], F32, tag=f"respf{h}",
                               name=f"respf{h}")
            nc.sync.dma_start(respf[:], resp_d.ap()[h * 128:(h + 1) * 128, :])
            respb = rpool.tile([128, L * UNITS], BF16, tag=f"respb{h}",
                               name=f"respb{h}")
            nc.scalar.copy(respb[:], respf[:])
            resps.append(respb)

        # ------------- Newton sparsemax on the 192-row shard -------------
        nst = cpool.tile([128, 8], F32, tag="nst")
        tau0 = cpool.tile([128, 1], F32, tag="tau0")
        tau1 = cpool.tile([64, 1], F32, tag="tau1")
        ntau0 = cpool.tile([128, 1], F32, tag="ntau0")
        ntau1 = cpool.tile([64, 1], F32, tag="ntau1")
        scr = cpool.tile([128, F], BF16, tag="nscr")
        scra = cpool.tile([128, F], BF16, tag="nscra")

        s0, k0 = nst[:, 0:1], nst[:, 1:2]
        rk0, d0 = nst[:, 2:3], nst[:, 3:4]
        s1, k1 = nst[0:64, 4:5], nst[0:64, 5:6]
        rk1, d1 = nst[0:64, 6:7], nst[0:64, 7:8]

        nc.vector.tensor_reduce(s0, zs0[:], AX.X, OP.add)
        nc.vector.tensor_scalar(tau0[:], s0, -1.0, 1.0 / F, OP.add, OP.mult)
        nc.vector.tensor_scalar(ntau0[:], tau0[:], -1.0, None, OP.mult)
        nc.vector.tensor_reduce(s1, zs1[:], AX.X, OP.add)
        nc.vector.tensor_scalar(tau1[:], s1, -1.0, 1.0 / F, OP.add, OP.mult)
        nc.vector.tensor_scalar(ntau1[:], tau1[:], -1.0, None, OP.mult)

        for _ in range(NEWTON_ITERS):
            nc.scalar.activation(scra[:], zs0[:], ACTF.Relu,
                                 bias=ntau0[:], accum_out=s0)
            nc.vector.tensor_scalar(scr[:], zs0[:], tau0[:], 0.0,
                                    OP.is_gt, OP.add, accum_out=k0)
            nc.vector.reciprocal(rk0, k0)
            nc.vector.tensor_scalar(d0, s0, -1.0, rk0, OP.add, OP.mult)
            nc.vector.scalar_tensor_tensor(tau0[:], d0, 1.0, tau0[:],
                                           OP.mult, OP.add)
            nc.vector.tensor_scalar(ntau0[:], tau0[:], -1.0, None, OP.mult)
            nc.scalar.activation(scra[0:64, :], zs1[:], ACTF.Relu,
                                 bias=ntau1[:], accum_out=s1)
            nc.vector.tensor_scalar(scr[0:64, :], zs1[:], tau1[:], 0.0,
                                    OP.is_gt, OP.add, accum_out=k1)
            nc.vector.reciprocal(rk1, k1)
            nc.vector.tensor_scalar(d1, s1, -1.0, rk1, OP.add, OP.mult)
            nc.vector.scalar_tensor_tensor(tau1[:], d1, 1.0, tau1[:],
                                           OP.mult, OP.add)
            nc.vector.tensor_scalar(ntau1[:], tau1[:], -1.0, None, OP.mult)

        nc.sync.dma_start(tau_in.ap()[0:128], tau0[:])
        nc.sync.dma_start(tau_in.ap()[128:RSH], tau1[:])
        nc.gpsimd.collective_compute(
            "AllGather", OP.bypass, replica_groups=RG,
            ins=[tau_in.ap()], outs=[tau_out.ap()])
        # gather tau transposed (12 partitions x 512 B lines), PE-transpose
        tauT = cpool.tile([NJ, 128], F32, tag="tauT")
        nc.sync.dma_start(tauT[:], tau_out.ap().rearrange("(j p) -> j p",
                                                          p=128))
        ptau = ps_g.tile([128, 512], F32, tag="psg", name="ptau")
        nc.tensor.transpose(ptau[:, 0:NJ], tauT[:], ident[0:NJ, 0:NJ])
        tauall = cpool.tile([128, NJ], F32, tag="tauall")
        ntauall = cpool.tile([128, NJ], F32, tag="ntauall")
        nc.scalar.copy(tauall[:], ptau[:, 0:NJ])
        nc.scalar.mul(ntauall[:], ptau[:, 0:NJ], -1.0)

        # ---------------- x^T via PE transposes ----------------
        inT = []
        inTb = [[None] * 4, [None] * 4]
        for h in range(2):
            t = cpool.tile([128, BC], F32, tag=f"inT{h}", name=f"inT{h}")
            inT.append(t)
        for i in range(4):
            for h in range(2):
                pt = ps_g.tile([128, 512], F32, tag="psg", name="pt")
                nc.tensor.transpose(pt[:, 0:128],
                                    ix[i][:, h * 128:(h + 1) * 128], ident[:])
                nc.scalar.copy(inT[h][:, i * 128:(i + 1) * 128], pt[:, 0:128])
                tb = cpool.tile([128, 128], BF16, tag=f"inTb{h}_{i}",
                                name=f"inTb{h}_{i}")
                nc.vector.tensor_copy(tb[:], inT[h][:, i * 128:(i + 1) * 128])
                inTb[h][i] = tb

        # ---------------- attention ----------------
        memT = []
        for h in range(2):
            t = cpool.tile([128, S], F32, tag=f"memT{h}", name=f"memT{h}")
            pt = ps_g.tile([128, 512], F32, tag="psg", name="pt")
            nc.tensor.transpose(pt[0:128, 0:M],
                                mem_sb[:, h * 128:(h + 1) * 128],
                                ident[0:M, 0:M])
            nc.gpsimd.memset(t[:, 0:1], 1.0)
            nc.scalar.copy(t[:, 1:S], pt[0:128, 0:M])
            memT.append(t)

        wk2, wv2 = [], []
        for h in range(2):
            for name, wsrc, dstl in (("k", wk_f, wk2), ("v", wv_f, wv2)):
                t = cpool.tile([128, S, HK], BF16, tag=f"w2{name}{h}",
                               name=f"w2{name}{h}")
                nc.vector.tensor_tensor(
                    t[:],
                    memT[h][:].unsqueeze(2).broadcast_to((128, S, HK)),
                    wsrc[:, h, :].unsqueeze(1).broadcast_to((128, S, HK)),
                    OP.mult)
                dstl.append(t)

        obb = []
        for i in range(4):
            xTc = [inTb[h][i][:] for h in range(2)]
            kvA = ps_at.tile([128, 512], F32, tag="kvA", name="kvA")
            for h in range(2):
                nc.tensor.matmul(kvA[:, 432:432 + HK], xTc[h], wq_b[:, h, :],
                                 start=(h == 0), stop=False)
            nc.tensor.matmul(kvA[:, 432:432 + HK],
                             ones1[:, i * 128:(i + 1) * 128],
                             bq_row[:], start=False, stop=True)
            kkp = kvA
            vvp = ps_at.tile([128, 512], F32, tag="kvB", name="kvB")
            for dst, w2, brow in ((kkp, wk2, bk_row), (vvp, wv2, bv_row)):
                for h in range(2):
                    nc.tensor.matmul(dst[:, 0:S * HK], xTc[h],
                                     w2[h][:].rearrange("p t k -> p (t k)"),
                                     start=(h == 0), stop=False)
                nc.tensor.matmul(dst[:, 0:S * HK],
                                 ones1[:, i * 128:(i + 1) * 128],
                                 brow[:], start=False, stop=True)

            q0s = apool.tile([128, HK], F32, tag="q0s")
            nc.scalar.copy(q0s[:], kvA[:, 432:432 + HK])
            prod = apool.tile([128, S, H, K], F32, tag="prod")
            nc.vector.tensor_tensor(
                prod[:],
                kkp[:, 0:S * HK].rearrange("p (t h k) -> p t h k", t=S, h=H),
                q0s[:].rearrange("p (h k) -> p h k", h=H).unsqueeze(1)
                    .broadcast_to((128, S, H, K)),
                OP.mult)
            sc_ht = apool.tile([128, H, S], F32, tag="scht")
            nc.vector.tensor_reduce(sc_ht[:].transpose([0, 2, 1]), prod[:],
                                    AX.X, OP.add)
            mx = apool.tile([128, H, 2], F32, tag="mx")
            nc.vector.tensor_reduce(mx[:, :, 0:1].squeeze(2), sc_ht[:],
                                    AX.X, OP.max)
            nc.vector.tensor_scalar(mx[:, :, 1:2], mx[:, :, 0:1],
                                    -(2.0 ** -0.5), None, OP.mult)
            ex = apool.tile([128, H, S], BF16, tag="ex")
            den = apool.tile([128, H, 2], F32, tag="den")
            for hh in range(H):
                nc.scalar.activation(ex[:, hh, :], sc_ht[:, hh, :], ACTF.Exp,
                                     bias=mx[:, hh, 1:2], scale=2.0 ** -0.5,
                                     accum_out=den[:, hh, 0:1])
            po = apool.tile([128, H, K, S], F32, tag="po")
            nc.vector.tensor_tensor(
                po[:].transpose([0, 3, 1, 2]),
                vvp[:, 0:S * HK].rearrange("p (t h k) -> p t h k", t=S, h=H),
                ex[:].transpose([0, 2, 1]).unsqueeze(3)
                    .broadcast_to((128, S, H, K)),
                OP.mult)
            ov = apool.tile([128, H, K], F32, tag="ov")
            nc.vector.tensor_reduce(ov[:], po[:], AX.X, OP.add)
            nc.vector.reciprocal(den[:, :, 1:2], den[:, :, 0:1])
            ob = apool.tile([128, HK], BF16, tag=f"ob{i}", name=f"ob{i}")
            nc.vector.tensor_tensor(
                ob[:].rearrange("p (h k) -> p h k", h=H), ov[:],
                den[:, :, 1:2].broadcast_to((128, H, K)), OP.mult)
            obb.append(ob)

        oTb = []
        for i in range(4):
            pt = ps_b.tile([128, 128], BF16, tag="ptrb", name="pt2")
            nc.tensor.transpose(pt[0:HK, 0:128], obb[i][:], identb[:])
            t = cpool.tile([HK, 128], BF16, tag=f"oTb{i}", name=f"oTb{i}")
            nc.scalar.copy(t[:], pt[0:HK, 0:128])
            oTb.append(t)

        xTb = []
        for h in range(2):
            xh = ps_g.tile([128, BC], F32, tag="psg", name="xh")
            for i in range(4):
                sl = slice(i * 128, (i + 1) * 128)
                nc.tensor.matmul(xh[:, sl], wo_b[:, h * 128:(h + 1) * 128],
                                 oTb[i][:], start=True, stop=False)
                nc.tensor.matmul(xh[:, sl], bo_b[:, h * 128:(h + 1) * 128],
                                 ones1[:, sl], start=False, stop=True)
            xb = cpool.tile([128, BC], BF16, tag=f"xTb{h}", name=f"xTb{h}")
            nc.vector.tensor_tensor(xb[:], inT[h][:], xh[:], OP.add)
            xTb.append(xb)
        if DEBUG:
            for h in range(2):
                nc.sync.dma_start(dbg_x.ap()[h], xTb[h][:])

        # -------------- sparsemax output + PE transposes --------------
        fs_t = cpool.tile([128, NJ, F], BF16, tag="fs")
        for j in range(NJ):
            nc.scalar.activation(fs_t[:, j, :], zt[:, j, :], ACTF.Relu,
                                 bias=ntauall[:, j:j + 1])
        if DEBUG:
            nc.sync.dma_start(dbg_tau.ap(), tauall[:])
        fsT = [[None] * NJ, [None] * NJ]
        for j in range(NJ):
            for h in range(2):
                pt = ps_b.tile([128, 128], BF16, tag="ptrb", name="pt2")
                nc.tensor.transpose(pt[:], fs_t[:, j, h * 128:(h + 1) * 128],
                                    identb[:])
                t = cpool.tile([128, 128], BF16, tag=f"fsT{h}_{j}",
                               name=f"fsT{h}_{j}")
                nc.scalar.copy(t[:], pt[:])
                fsT[h][j] = t

        sv = cpool.tile([128, NJ], F32, tag="sv")
        cv = cpool.tile([128, NJ], F32, tag="cv")
        lt_flat = lt_t[:].rearrange("p d h -> p (d h)")
        th_flat = th_t[:].rearrange("p d h -> p (d h)")
        nc.scalar.activation(sv[:], lt_flat, ACTF.Exp, scale=-1.0)
        nc.vector.tensor_scalar(sv[:], sv[:], 0.5, None, OP.mult)
        nc.vector.tensor_tensor(cv[:], th_flat, sv[:], OP.mult)
        nc.vector.tensor_scalar(cv[:], cv[:], -1.0, 0.5, OP.mult, OP.add)

        # ---------------- feat + bins ----------------
        # f01[:, b, j, :]: b=0 -> f0 = 1-bins, b=1 -> f1 = bins
        f01 = fpool.tile([128, 2, NJ, BC], BF16, tag="f01")
        for j in range(NJ):
            ft = ps_g.tile([128, BC], F32, tag="psg", name="ft")
            for h in range(2):
                nc.tensor.matmul(ft[:], fsT[h][j][:], xTb[h][:],
                                 start=(h == 0), stop=(h == 1))
            aj = apool.tile([128, BC], BF16, tag="aj")
            nc.scalar.activation(aj[:], ft[:], ACTF.Relu,
                                 bias=cv[:, j:j + 1], scale=sv[:, j:j + 1])
            nc.vector.tensor_scalar(f01[:, 1, j, :], aj[:], 1.0, None, OP.min)
            nc.scalar.activation(f01[:, 0, j, :], aj[:], ACTF.Relu,
                                 bias=1.0, scale=-1.0)
        if DEBUG:
            nc.sync.dma_start(dbg_f1.ap(), f01[:, 1])

        # ------------- kronecker + tree contraction -------------
        accs = [ps_a.tile([UNITS, BC], F32, tag=f"acc{a}", name=f"acc{a}")
                for a in range(2)]
        for h in range(2):
            respb = resps[h]

            def fsel(d):
                # [128, 2(bit), BC] for depth d, this tree half
                return f01[:, :, d * 2 + h, :]

            hi2 = hpool.tile([128, 2, 2, BC], BF16, tag="hi2")
            lo2 = hpool.tile([128, 2, 2, BC], BF16, tag="lo2")
            hi = hpool.tile([128, 4, 2, BC], BF16, tag="hi")
            lo = hpool.tile([128, 4, 2, BC], BF16, tag="lo")
            nc.vector.tensor_tensor(
                hi2[:], fsel(5).unsqueeze(2).broadcast_to((128, 2, 2, BC)),
                fsel(4).unsqueeze(1).broadcast_to((128, 2, 2, BC)), OP.mult)
            nc.vector.tensor_tensor(
                lo2[:], fsel(2).unsqueeze(2).broadcast_to((128, 2, 2, BC)),
                fsel(1).unsqueeze(1).broadcast_to((128, 2, 2, BC)), OP.mult)
            hi2f = hi2[:].rearrange("p a b c -> p (a b) c")
            lo2f = lo2[:].rearrange("p a b c -> p (a b) c")
            nc.vector.tensor_tensor(
                hi[:], hi2f.unsqueeze(2).broadcast_to((128, 4, 2, BC)),
                fsel(3).unsqueeze(1).broadcast_to((128, 4, 2, BC)), OP.mult)
            nc.vector.tensor_tensor(
                lo[:], lo2f.unsqueeze(2).broadcast_to((128, 4, 2, BC)),
                fsel(0).unsqueeze(1).broadcast_to((128, 4, 2, BC)), OP.mult)
            hif = hi[:].rearrange("p a b c -> p (a b) c")
            lof = lo[:].rearrange("p a b c -> p (a b) c")
            for i in range(8):
                leaf = lpool.tile([128, 8, BC], BF16, tag="leaf")
                nc.vector.tensor_tensor(
                    leaf[:], hif[:, i:i + 1, :].broadcast_to((128, 8, BC)),
                    lof, OP.mult)
                for g in range(8):
                    l = i * 8 + g
                    nc.tensor.matmul(accs[l % 2][:],
                                     respb[:, l * UNITS:(l + 1) * UNITS],
                                     leaf[:, g, :],
                                     start=(h == 0 and l < 2),
                                     stop=(h == 1 and l >= L - 2))

        # ---------------- output ----------------
        outT = opool.tile([UNITS, BC], F32, tag="outT")
        nc.vector.tensor_copy(outT[:], accs[0][:])
        nc.vector.tensor_tensor(outT[:], outT[:], accs[1][:], OP.add)
        for i in range(4):
            pt = ps_g.tile([128, 512], F32, tag="psg", name="pt")
            nc.tensor.transpose(pt[:, 0:UNITS], outT[:, i * 128:(i + 1) * 128],
                                ident[0:UNITS, 0:UNITS])
            ysb = opool.tile([128, UNITS], F32, tag="ysb")
            nc.scalar.copy(ysb[:], pt[:, 0:UNITS])
            nc.sync.dma_start(y_d.ap()[i * 128:(i + 1) * 128, :], ysb[:])

    nc.compile()
    return nc


_CACHED = None


def _get_program():
    global _CACHED
    if _CACHED is None:
        _CACHED = _build_program()
    return _CACHED


def _make_in_maps(inputs, memory, Wq, bq, Wk, bk, Wv, bv, Wo, bo,
                  fs_logits, thresholds, log_temp, response):
    f32 = np.float32
    z_dmaj = np.ascontiguousarray(
        np.asarray(fs_logits, f32).transpose(1, 0, 2).reshape(ROWS, F))
    common = {
        "z_all": z_dmaj,
        "mem": np.ascontiguousarray(np.asarray(memory, f32)),
        "wq": np.ascontiguousarray(np.asarray(Wq, f32).reshape(F, HK)),
        "wk": np.ascontiguousarray(np.asarray(Wk, f32).reshape(F, HK)),
        "wv": np.ascontiguousarray(np.asarray(Wv, f32).reshape(F, HK)),
        "bq": np.ascontiguousarray(np.asarray(bq, f32).reshape(HK)),
        "bk": np.ascontiguousarray(np.asarray(bk, f32).reshape(HK)),
        "bv": np.ascontiguousarray(np.asarray(bv, f32).reshape(HK)),
        "wo": np.ascontiguousarray(np.asarray(Wo, f32).reshape(HK, F)),
        "bo": np.ascontiguousarray(np.asarray(bo, f32).reshape(F)),
        "th": np.ascontiguousarray(np.asarray(thresholds, f32)),
        "lt": np.ascontiguousarray(np.asarray(log_temp, f32)),
        "resp": np.ascontiguousarray(
            np.asarray(response, f32).reshape(NT, L * UNITS)),
    }
    xs = np.ascontiguousarray(np.asarray(inputs, f32))
    in_maps = []
    for c in range(NCORES):
        m = dict(common)
        m["x_in"] = np.ascontiguousarray(xs[c * BC:(c + 1) * BC])
        m["z_sh"] = np.ascontiguousarray(z_dmaj[c * RSH:(c + 1) * RSH])
        in_maps.append(m)
    return in_maps


def run(inputs_dict, trace=False):
    nc = _get_program()
    in_maps = _make_in_maps(**inputs_dict)
    res = run_bass_kernel_spmd(nc, in_maps, list(range(NCORES)), trace=trace)
    out = np.concatenate([res.results[c]["y"] for c in range(NCORES)], axis=0)
    return out.astype(np.float32), res


def kernel(inputs, memory, Wq, bq, Wk, bk, Wv, bv, Wo, bo,
           fs_logits, thresholds, log_temp, response):
    out, _ = run(dict(
        inputs=inputs, memory=memory, Wq=Wq, bq=bq, Wk=Wk, bk=bk,
        Wv=Wv, bv=bv, Wo=Wo, bo=bo, fs_logits=fs_logits,
        thresholds=thresholds, log_temp=log_temp, response=response))
    return out
